# revision 3
# baseline (speedup 1.0000x reference)
"""Trainium2 Bass kernel for nn_CombinedModel_wGCN (GNN message passing).

Reference computation per event b (B=4096 events, N=128 particles):
  x = concat(feat, emb_table[pdg])          [128, 16]
  x = x @ W_in + b_in                       [128, 128]
  6x: x = relu(x @ W_h[l] + b_h[l]); x = adj @ x
  out[b] = (mask-weighted mean_i x) @ W_out + b_out

Strategy (pure data-parallel over 8 cores, 512 events each):
  - All state kept transposed: Xh = x^T [d, i] so the dense layer is
    matmul(lhsT=W[l] (stationary, f32r), rhs=Xh batched 4 events [d, 512])
    at full PE rate; bias+relu fused into one ACT op (bias per-partition).
  - Aggregation adj @ R runs in bf16 (error ~1e-4): per event
    matmul(lhsT=R[j,d], rhs=adjT[j,i']); adjT is pre-transposed and cast
    on the host. R is produced by a PE transpose of the relu output.
  - Masked-mean pooling is folded into v = adj^T (mask/denom) computed on
    the host, so the last aggregation collapses to an N=1 matmul per event
    accumulated into a persistent PSUM bank; final W_out projection is one
    f32r matmul over all 512 pooled columns.
"""

import os
import numpy as np
import ml_dtypes

B, N = 4096, 128
NUM_FEAT, EMBED = 8, 8
UNITS = 128
HIDDEN = 6
NCORES = 8
BC = B // NCORES  # events per core
G = 4  # events per group (one PSUM bank of 512 f32 columns)
NG = BC // G
D0 = NUM_FEAT + EMBED + 1  # input features augmented with ones row (bias)

_cache = {}


def _build_nc(ngroups):
    import concourse.tile as tile
    from concourse import mybir, bacc

    f32 = mybir.dt.float32
    f32r = mybir.dt.float32r
    bf16 = mybir.dt.bfloat16
    Relu = mybir.ActivationFunctionType.Relu

    nc = bacc.Bacc(
        trn_type="TRN2", target_bir_lowering=False, debug=False, num_devices=NCORES
    )
    d_adjt = nc.declare_dram_parameter("adjt", [NG, 128, G * 128], bf16, isOutput=False)
    d_x0t = nc.declare_dram_parameter("x0t", [NG, D0, G * 128], f32, isOutput=False)
    d_vt = nc.declare_dram_parameter("vt", [128, BC], bf16, isOutput=False)
    d_wh = nc.declare_dram_parameter("wh", [HIDDEN, 128, 128], f32, isOutput=False)
    d_bh = nc.declare_dram_parameter("bh", [HIDDEN, 128], f32, isOutput=False)
    d_win = nc.declare_dram_parameter("win", [D0, 128], f32, isOutput=False)
    d_wout = nc.declare_dram_parameter("wout", [128, 1], f32, isOutput=False)
    d_bout = nc.declare_dram_parameter("bout", [1, 1], f32, isOutput=False)
    d_ident = nc.declare_dram_parameter("ident", [128, 128], bf16, isOutput=False)
    d_out = nc.declare_dram_parameter("out", [1, BC], f32, isOutput=True)

    with tile.TileContext(nc) as tc:
        with (
            tc.tile_pool(name="const", bufs=1) as constp,
            tc.tile_pool(name="stage", bufs=2) as stagep,
            tc.tile_pool(name="adj", bufs=4) as adjp,
            tc.tile_pool(name="x0", bufs=4) as x0p,
            tc.tile_pool(name="work", bufs=3) as workp,
            tc.tile_pool(name="ps", bufs=2, space="PSUM") as psp,
            tc.tile_pool(name="psfix", bufs=1, space="PSUM") as psfixp,
        ):
            # ---- constants ----
            whr = []
            bhc = []
            for l in range(HIDDEN):
                wst = stagep.tile([128, 128], f32, tag="wstage")
                nc.sync.dma_start(wst[:], d_wh[l])
                wr = constp.tile([128, 128], f32r, tag=f"whr{l}")
                nc.vector.tensor_copy(wr[:], wst[:])
                whr.append(wr)
                bc = constp.tile([128, 1], f32, tag=f"bh{l}")
                nc.sync.dma_start(bc[:], d_bh[l].rearrange("(d o) -> d o", o=1))
                bhc.append(bc)
            winst = stagep.tile([D0, 128], f32, tag="winstage")
            nc.sync.dma_start(winst[:], d_win[:])
            winr = constp.tile([D0, 128], f32r, tag="winr")
            nc.vector.tensor_copy(winr[:], winst[:])
            woutst = stagep.tile([128, 1], f32, tag="woutstage")
            nc.sync.dma_start(woutst[:], d_wout[:])
            woutr = constp.tile([128, 1], f32r, tag="woutr")
            nc.vector.tensor_copy(woutr[:], woutst[:])
            boutt = constp.tile([1, 1], f32, tag="bout")
            nc.sync.dma_start(boutt[:], d_bout[:])
            ident = constp.tile([128, 128], bf16, tag="ident")
            nc.sync.dma_start(ident[:], d_ident[:])
            vsb = constp.tile([128, BC], bf16, tag="vsb")
            nc.sync.dma_start(vsb[:], d_vt[:])

            pooled = psfixp.tile([128, BC], f32, tag="pooled")

            # ---- main loop over groups of G events ----
            for g in range(ngroups):
                adjt = adjp.tile([128, G * 128], bf16, tag="adjt")
                nc.sync.dma_start(adjt[:], d_adjt[g])
                x0t = x0p.tile([D0, G * 128], f32, tag="x0t")
                nc.sync.dma_start(x0t[:], d_x0t[g])
                x0r = workp.tile([D0, G * 128], f32r, tag="x0r")
                nc.vector.tensor_copy(x0r[:], x0t[:])

                # layer_in: x @ W_in + b_in (no relu); bias via ones-row aug
                pin = psp.tile([128, G * 128], f32, tag="dense")
                nc.tensor.matmul(pin[:], winr[:], x0r[:], start=True, stop=True)
                xh = workp.tile([128, G * 128], f32r, tag="xh")
                nc.vector.tensor_copy(xh[:], pin[:])

                for l in range(HIDDEN):
                    # dense: P[d_out, i] = W[l]^T @ Xh (+bias, relu below)
                    pd = psp.tile([128, G * 128], f32, tag="dense")
                    nc.tensor.matmul(pd[:], whr[l][:], xh[:], start=True, stop=True)
                    rt = workp.tile([128, G * 128], bf16, tag="rt")
                    nc.scalar.activation(rt[:], pd[:], Relu, bias=bhc[l][:])
                    # transpose R^T[d,j] -> R[j,d] per event (PE, bf16)
                    pr = psp.tile([128, G * 128], bf16, tag="rpsum")
                    for e in range(G):
                        s = slice(e * 128, (e + 1) * 128)
                        nc.tensor.transpose(pr[:, s], rt[:, s], ident[:])
                    r = workp.tile([128, G * 128], bf16, tag="r")
                    if l % 2 == 0:
                        nc.vector.tensor_copy(r[:], pr[:])
                    else:
                        nc.scalar.copy(r[:], pr[:])
                    if l < HIDDEN - 1:
                        # aggregation: X'[d, i'] = R^T @ adjT per event
                        pa = psp.tile([128, G * 128], f32, tag="agg")
                        for e in range(G):
                            s = slice(e * 128, (e + 1) * 128)
                            nc.tensor.matmul(
                                pa[:, s], r[:, s], adjt[:, s], start=True, stop=True
                            )
                        xh = workp.tile([128, G * 128], f32r, tag="xh")
                        nc.vector.tensor_copy(xh[:], pa[:])
                    else:
                        # pooled[:, ev] = R^T @ v_ev  (masked mean folded into v)
                        for e in range(G):
                            s = slice(e * 128, (e + 1) * 128)
                            ev = g * G + e
                            nc.tensor.matmul(
                                pooled[:, ev : ev + 1],
                                r[:, s],
                                vsb[:, ev : ev + 1],
                                start=True,
                                stop=True,
                            )

            # ---- final projection: out = pooled^T @ W_out + b_out ----
            pooled_r = constp.tile([128, BC], f32r, tag="pooledr")
            nc.vector.tensor_copy(pooled_r[:], pooled[:])
            pout = psfixp.tile([1, BC], f32, tag="pout")
            nc.tensor.matmul(pout[:], woutr[:], pooled_r[:], start=True, stop=True)
            outsb = constp.tile([1, BC], f32, tag="outsb")
            nc.vector.tensor_scalar_add(outsb[:], pout[:], boutt[:])
            nc.sync.dma_start(d_out[:], outsb[:])

    nc.finalize()
    return nc


def _prep_inputs(pdg, feat, adj, mask, emb_table, W_in, b_in, W_h, b_h, W_out, b_out):
    bf = ml_dtypes.bfloat16
    pdg = np.asarray(pdg)
    feat = np.asarray(feat, dtype=np.float32)
    adj = np.asarray(adj, dtype=np.float32)
    mask = np.asarray(mask, dtype=np.float32)
    emb_table = np.asarray(emb_table, dtype=np.float32)

    emb = emb_table[pdg]  # [B, N, EMBED]
    ones = np.ones((B, N, 1), dtype=np.float32)
    x0 = np.concatenate([feat, emb, ones], axis=-1)  # [B, N, 17]
    x0t = np.ascontiguousarray(x0.transpose(0, 2, 1))  # [B, 17, N]
    # blocked by groups of G events: [B/G, 17, G*N]
    x0t4 = np.ascontiguousarray(
        x0t.reshape(B // G, G, D0, N).transpose(0, 2, 1, 3)
    ).reshape(B // G, D0, G * N)

    adjt = adj.transpose(0, 2, 1).astype(bf)  # [B, j, i]
    adjt4 = np.ascontiguousarray(
        adjt.reshape(B // G, G, N, N).transpose(0, 2, 1, 3)
    ).reshape(B // G, N, G * N)

    denom = np.clip(mask.sum(axis=1, keepdims=True), 1.0, None)
    m_scaled = (mask / denom).astype(np.float32)  # [B, N]
    v = np.matmul(m_scaled[:, None, :], adj).squeeze(1)  # [B, N] v[b,j]
    vt = v.T.astype(bf)  # [N, B] column per event

    win_aug = np.concatenate(
        [np.asarray(W_in, np.float32), np.asarray(b_in, np.float32)[None, :]], axis=0
    )  # [17, 128]
    in_maps = []
    for c in range(NCORES):
        ev = slice(c * BC, (c + 1) * BC)
        gv = slice(c * (BC // G), (c + 1) * (BC // G))
        in_maps.append(
            {
                "adjt": adjt4[gv],
                "x0t": x0t4[gv],
                "vt": np.ascontiguousarray(vt[:, ev]),
                "wh": np.asarray(W_h, np.float32),
                "bh": np.asarray(b_h, np.float32),
                "win": win_aug,
                "wout": np.asarray(W_out, np.float32),
                "bout": np.asarray(b_out, np.float32).reshape(1, 1),
                "ident": np.eye(128, dtype=bf),
            }
        )
    return in_maps


def kernel(pdg, feat, adj, mask, emb_table, W_in, b_in, W_h, b_h, W_out, b_out):
    from concourse.bass_utils import run_bass_kernel_spmd

    ngroups = int(os.environ.get("KERNEL_NGROUPS", NG))
    key = ("nc", ngroups)
    if key not in _cache:
        _cache[key] = _build_nc(ngroups)
    nc = _cache[key]

    in_maps = _prep_inputs(
        pdg, feat, adj, mask, emb_table, W_in, b_in, W_h, b_h, W_out, b_out
    )
    trace = bool(int(os.environ.get("KERNEL_TRACE", "0")))
    if trace:
        try:
            tmpdir = os.environ.get("KERNEL_TRACE_DIR") or None
            res = run_bass_kernel_spmd(
                nc, in_maps, core_ids=list(range(NCORES)), trace=True, tmpdir=tmpdir
            )
            _cache["last_exec_time_ns"] = res.exec_time_ns
            _cache["last_results"] = res
        except Exception as e:
            print(f"trace run failed ({type(e).__name__}: {e}); rerunning untraced")
            _cache["last_exec_time_ns"] = None
            res = run_bass_kernel_spmd(nc, in_maps, core_ids=list(range(NCORES)))
    else:
        res = run_bass_kernel_spmd(nc, in_maps, core_ids=list(range(NCORES)))
    out = np.concatenate([res.results[c]["out"].reshape(BC) for c in range(NCORES)])
    return out.reshape(B, 1).astype(np.float32)
